# revision 6
# baseline (speedup 1.0000x reference)
"""Trainium2 Bass kernel for multi-graph SpMM propagation (GNN message passing).

Computation (per graph): f0 = feat; f_{l+1} = segsum(vals * f_l[cols], rows);
kernel outputs raw f1, f2 shards (fp16). Host applies l2norm + layer average.

Strategy: 1D row-partition each graph across 8 cores; one identical SPMD
program for all cores (per-(block,bucket) tile counts padded to the max over
cores). Edge tiles of 128; per tile:
  - dma_gather (256B fp32 rows) fetches feat[col]; int16 indices are relative
    to a 32768-row column bucket window; calls cover <=8 tiles (SWDGE ring cap)
  - ScalarE casts gathered fp32 -> fp16 into a per-segment buffer
  - VectorE dual-op tensor_scalar builds a val-weighted one-hot vs iota const
  - PE matmul (fp16, FWL) accumulates onehot.T @ gathered into the block PSUM
Blocks are grouped into segments (16 blocks); gathers are bucket-major within
a segment, matmuls block-major. Layer boundary: fp16 AllGather per graph.
"""

import numpy as np

import concourse.bacc as bacc
import concourse.bass as bass
import concourse.mybir as mybir
import concourse.tile as tile
from concourse.bass_utils import run_bass_kernel_spmd

NCORES = 8
P = 128
D = 64
BUCKET = 32768       # int16-addressable window of table rows per gather call
CALL_TILES = 8       # tiles per dma_gather call (1024 idx = SWDGE ring cap)
SEG_BLOCKS = 16      # blocks per segment

U, NI, NB = 100000, 50000, 20000


def _preprocess_graph(rows, cols, vals, n_total):
    rows = np.asarray(rows, dtype=np.int64)
    cols = np.asarray(cols, dtype=np.int64)
    vals = np.asarray(vals, dtype=np.float32)

    S = ((n_total + NCORES * P - 1) // (NCORES * P)) * P
    nblocks = S // P
    nbuckets = (NCORES * S + BUCKET - 1) // BUCKET
    nsegs = (nblocks + SEG_BLOCKS - 1) // SEG_BLOCKS

    core = np.minimum(rows // S, NCORES - 1)
    blk = (rows - core * S) // P
    bkt = cols // BUCKET
    seg = blk // SEG_BLOCKS

    # group id in slot order: (seg, bucket, block); per-core counts
    grp = (seg * nbuckets + bkt) * nblocks + blk
    ngrp = nsegs * nbuckets * nblocks
    counts = np.zeros((NCORES, ngrp), np.int64)
    np.add.at(counts, (core, grp), 1)

    tiles_per_grp = -(-counts.max(axis=0) // P)  # [ngrp]
    # Every block must produce >=1 tile so its PSUM/output gets written.
    tpg3 = tiles_per_grp.reshape(nsegs, nbuckets, nblocks)
    blk_tiles = tpg3.sum(axis=(0, 1))
    for b in np.where(blk_tiles == 0)[0]:
        tpg3[b // SEG_BLOCKS, 0, b] = 1
    tiles_per_grp = tpg3.reshape(-1)

    grp_tile_start = np.concatenate([[0], np.cumsum(tiles_per_grp)])
    T_tot = int(grp_tile_start[-1])

    # place edges into slots (rank within each (grp, core) run)
    key = grp * NCORES + core
    order = np.argsort(key, kind="stable")
    key_s = key[order]
    run_start = np.concatenate([[0], np.cumsum(counts.reshape(NCORES, ngrp).T.reshape(-1))])
    rank = np.arange(len(rows)) - run_start[key_s]
    slot = grp_tile_start[grp[order]] * P + rank
    c_s = core[order]

    col_rel = np.zeros((NCORES, T_tot * P), np.int16)
    rl_slots = np.zeros((NCORES, T_tot * P), np.float32)
    val_slots = np.zeros((NCORES, T_tot * P), np.float32)
    col_rel[c_s, slot] = (cols[order] - bkt[order] * BUCKET).astype(np.int16)
    rl_slots[c_s, slot] = (rows[order] - c_s * S - blk[order] * P).astype(np.float32)
    val_slots[c_s, slot] = vals[order].astype(np.float32)

    # schedules
    segs = []
    idx_chunks = []  # per-call [NCORES, 16, nt*8] int16
    call_col_off = 0
    col_rel_t = col_rel.reshape(NCORES, T_tot, P)
    for s in range(nsegs):
        b_lo = s * SEG_BLOCKS
        b_hi = min(nblocks, b_lo + SEG_BLOCKS)
        seg_t0 = int(grp_tile_start[(s * nbuckets + 0) * nblocks + b_lo])
        calls = []
        for B in range(nbuckets):
            g0 = (s * nbuckets + B) * nblocks
            t0 = int(grp_tile_start[g0 + b_lo])
            t1 = int(grp_tile_start[g0 + b_hi - 1] + tiles_per_grp[g0 + b_hi - 1])
            t = t0
            while t < t1:
                nt = min(CALL_TILES, t1 - t)
                calls.append(dict(bucket=B, t0=t, nt=nt, col_off=call_col_off))
                chunk = col_rel_t[:, t : t + nt, :].reshape(NCORES, nt * P)
                wrapped = np.zeros((NCORES, 16, nt * 8), np.int16)
                i = np.arange(nt * P)
                wrapped[:, i % 16, i // 16] = chunk
                idx_chunks.append(wrapped)
                call_col_off += nt * 8
                t += nt
        seg_t1 = t if calls else seg_t0
        blocks = []
        for b in range(b_lo, b_hi):
            tl = []
            for B in range(nbuckets):
                g = (s * nbuckets + B) * nblocks + b
                tl.extend(range(int(grp_tile_start[g]),
                                int(grp_tile_start[g]) + int(tiles_per_grp[g])))
            blocks.append((b, tl))
        segs.append(dict(seg_t0=seg_t0, seg_t1=seg_t1, calls=calls, blocks=blocks))

    idx_all = np.concatenate(idx_chunks, axis=2)  # [NCORES, 16, cols]
    idx_all = np.tile(idx_all, (1, 8, 1))         # replicate to 128 partitions

    def to_tiles(a):
        return np.ascontiguousarray(a.reshape(NCORES, T_tot, P).transpose(0, 2, 1))

    return dict(
        n_total=n_total, S=S, nblocks=nblocks, nbuckets=nbuckets,
        T_tot=T_tot, segs=segs, idx_ncols=idx_all.shape[2],
        idx=np.ascontiguousarray(idx_all),
        rl=to_tiles(rl_slots), val=to_tiles(val_slots),
    )


def _build_program(metas, graph_order):
    f16 = mybir.dt.float16
    f32 = mybir.dt.float32
    i16 = mybir.dt.int16

    nc = bacc.Bacc(
        "TRN2", target_bir_lowering=False, debug=False,
        enable_asserts=False, num_devices=NCORES,
    )

    tabs, idxs, rls, vls, f1s, f2s = {}, {}, {}, {}, {}, {}
    for g in graph_order:
        m = metas[g]
        tabs[g] = nc.dram_tensor(f"table_{g}", [m["n_total"], D], f32, kind="ExternalInput")
        idxs[g] = nc.dram_tensor(f"idx_{g}", [P, m["idx_ncols"]], i16, kind="ExternalInput")
        rls[g] = nc.dram_tensor(f"rl_{g}", [P, m["T_tot"]], f32, kind="ExternalInput")
        vls[g] = nc.dram_tensor(f"val_{g}", [P, m["T_tot"]], f32, kind="ExternalInput")
        f1s[g] = nc.dram_tensor(f"f1_{g}", [m["S"], D], f16, kind="ExternalOutput")
        f2s[g] = nc.dram_tensor(f"f2_{g}", [m["S"], D], f16, kind="ExternalOutput")

    seg_tiles_max = max(
        max(s["seg_t1"] - s["seg_t0"] for s in metas[g]["segs"]) for g in graph_order
    )
    seg_cols_max = max(
        max(max(sum(c["nt"] for c in s["calls"]) * 8, 1) for s in metas[g]["segs"])
        for g in graph_order
    )

    with tile.TileContext(nc) as tc:
        with (
            tc.tile_pool(name="const", bufs=1) as cpool,
            tc.tile_pool(name="meta", bufs=1) as mpool,
            tc.tile_pool(name="idxp", bufs=2) as ipool,
            tc.tile_pool(name="exp16", bufs=2) as e16pool,
            tc.tile_pool(name="exp32", bufs=2) as e32pool,
            tc.tile_pool(name="segp", bufs=2) as spool,
            tc.tile_pool(name="tmpp", bufs=4) as tpool,
            tc.tile_pool(name="oneh", bufs=12) as opool,
            tc.tile_pool(name="fout", bufs=6) as fpool,
            tc.tile_pool(name="psum", bufs=4, space="PSUM") as ppool,
            tc.tile_pool(name="dram", bufs=1, space="DRAM") as dpool,
        ):
            iota_t = cpool.tile([P, P], i16, name="iota_t")
            nc.gpsimd.iota(iota_t[:], pattern=[[1, P]], base=0, channel_multiplier=0)

            rl_sb, val_sb, ag_in, table2, table2f32 = {}, {}, {}, {}, {}
            for g in graph_order:
                m = metas[g]
                rl_sb[g] = mpool.tile([P, m["T_tot"]], f32, tag=f"rl_{g}", name=f"rl_sb_{g}")
                val_sb[g] = mpool.tile([P, m["T_tot"]], f32, tag=f"val_{g}", name=f"val_sb_{g}")
                nc.sync.dma_start(rl_sb[g][:], rls[g][:])
                nc.sync.dma_start(val_sb[g][:], vls[g][:])
                ag_in[g] = dpool.tile([m["S"], D], f16, tag=f"agin_{g}", name=f"ag_in_{g}")
                table2[g] = dpool.tile([m["S"] * NCORES, D], f16, tag=f"tab2_{g}", name=f"table2_{g}")
                table2f32[g] = dpool.tile([m["S"] * NCORES, D], f32, tag=f"tab2f_{g}", name=f"table2f32_{g}")

            def emit_layer(g, table_ap_fn, dst_drams):
                m = metas[g]
                for s in m["segs"]:
                    seg_t0 = s["seg_t0"]
                    segbuf = None
                    if s["calls"]:
                        ncols = sum(c["nt"] for c in s["calls"]) * 8
                        idxchunk = ipool.tile([P, seg_cols_max], i16, tag="idxc", name="idxchunk")
                        c0 = s["calls"][0]["col_off"]
                        nc.sync.dma_start(idxchunk[:, :ncols], idxs[g][:, c0 : c0 + ncols])
                        segbuf = spool.tile([P, seg_tiles_max, D], f16, tag="segbuf", name="segbuf")
                        for call in s["calls"]:
                            nt = call["nt"]
                            tmp = tpool.tile([P, CALL_TILES, D], f32, tag="gtmp", name="gtmp")
                            nc.gpsimd.dma_gather(
                                out_ap=tmp[:, :nt, :],
                                in_ap=table_ap_fn(call["bucket"]),
                                idxs_ap=idxchunk[:, call["col_off"] - c0 : call["col_off"] - c0 + nt * 8],
                                num_idxs=nt * P,
                                num_idxs_reg=nt * P,
                                elem_size=D,
                            )
                            so = call["t0"] - seg_t0
                            nc.scalar.activation(
                                segbuf[:, so : so + nt, :], tmp[:, :nt, :],
                                mybir.ActivationFunctionType.Copy,
                            )
                    for b, tl in s["blocks"]:
                        psum = ppool.tile([P, D], f32, tag="psum", name="psum")
                        for i, t in enumerate(tl):
                            onehot = opool.tile([P, P], f16, tag="oneh", name="onehot")
                            nc.vector.tensor_scalar(
                                onehot[:], iota_t[:],
                                rl_sb[g][:, t : t + 1], val_sb[g][:, t : t + 1],
                                mybir.AluOpType.is_equal, mybir.AluOpType.mult,
                            )
                            nc.tensor.matmul(
                                out=psum[:], lhsT=onehot[:],
                                rhs=segbuf[:, t - seg_t0, :],
                                start=(i == 0), stop=(i == len(tl) - 1),
                            )
                        f16t = fpool.tile([P, D], f16, tag="fout", name="f16t")
                        nc.scalar.activation(f16t[:], psum[:], mybir.ActivationFunctionType.Copy)
                        for dd in dst_drams:
                            nc.sync.dma_start(dd[b * P : (b + 1) * P, :], f16t[:])

            RW = 32  # table2 fp16->fp32 expansion: rows per partition per chunk

            def emit_expand(g):
                m = metas[g]
                nrows = m["S"] * NCORES
                r0 = 0
                while r0 < nrows:
                    rw = min(RW, (nrows - r0) // P)
                    ch = rw * D
                    sb16 = e16pool.tile([P, RW * D], f16, tag="e16", name="sb16")
                    sb32 = e32pool.tile([P, RW * D], f32, tag="e32", name="sb32")
                    src16 = table2[g][r0 : r0 + P * rw, :].rearrange(
                        "(p r) d -> p (r d)", p=P)
                    dst32 = table2f32[g][r0 : r0 + P * rw, :].rearrange(
                        "(p r) d -> p (r d)", p=P)
                    nc.sync.dma_start(sb16[:, :ch], src16)
                    nc.vector.tensor_copy(sb32[:, :ch], sb16[:, :ch])
                    nc.sync.dma_start(dst32, sb32[:, :ch])
                    r0 += P * rw

            rg = [list(range(NCORES))]
            for g in graph_order:
                emit_layer(g, lambda B, g=g: tabs[g][B * BUCKET :, :], [ag_in[g], f1s[g]])
                nc.gpsimd.collective_compute(
                    "AllGather", mybir.AluOpType.bypass, replica_groups=rg,
                    ins=[ag_in[g][:]], outs=[table2[g][:]],
                )
            for g in graph_order:
                emit_expand(g)
            for g in graph_order:
                emit_layer(g, lambda B, g=g: table2f32[g][B * BUCKET :, :], [f2s[g]])

    nc.compile()
    return nc


def _l2norm_rows(x):
    x = x.astype(np.float32)
    n = np.sqrt(np.sum(x * x, axis=1, keepdims=True))
    return x / np.maximum(n, 1e-12)


def _make_in_maps(graphs, graph_order, metas):
    in_maps = []
    for k in range(NCORES):
        im = {}
        for g in graph_order:
            feat = graphs[g][0]
            m = metas[g]
            im[f"table_{g}"] = np.ascontiguousarray(feat.astype(np.float32))
            im[f"idx_{g}"] = np.ascontiguousarray(m["idx"][k])
            im[f"rl_{g}"] = np.ascontiguousarray(m["rl"][k])
            im[f"val_{g}"] = np.ascontiguousarray(m["val"][k])
        in_maps.append(im)
    return in_maps


def _run(graphs, graph_order, run_fn=None):
    metas = {g: _preprocess_graph(graphs[g][1], graphs[g][2], graphs[g][3],
                                  graphs[g][0].shape[0]) for g in graph_order}
    nc = _build_program(metas, graph_order)
    in_maps = _make_in_maps(graphs, graph_order, metas)

    if run_fn is None:
        results = run_bass_kernel_spmd(nc, in_maps, core_ids=list(range(NCORES))).results
    else:
        results = run_fn(nc, in_maps)

    out = {}
    for g in graph_order:
        m = metas[g]
        n = m["n_total"]
        f1 = np.concatenate([results[k][f"f1_{g}"] for k in range(NCORES)], axis=0)
        f2 = np.concatenate([results[k][f"f2_{g}"] for k in range(NCORES)], axis=0)
        out[g] = (f1[:n].astype(np.float32), f2[:n].astype(np.float32))
    return out


def kernel(
    users_feature, items_feature, bundles_feature,
    ui_vals, bi_vals, ub_vals,
    ui_rows, ui_cols, bi_rows, bi_cols, ub_rows, ub_cols,
):
    users_feature = np.asarray(users_feature, dtype=np.float32)
    items_feature = np.asarray(items_feature, dtype=np.float32)
    bundles_feature = np.asarray(bundles_feature, dtype=np.float32)

    feats = {
        "ui": np.concatenate([users_feature, items_feature], axis=0),
        "bi": np.concatenate([bundles_feature, items_feature], axis=0),
        "ub": np.concatenate([users_feature, bundles_feature], axis=0),
    }
    graphs = {
        "ui": (feats["ui"], ui_rows, ui_cols, ui_vals),
        "bi": (feats["bi"], bi_rows, bi_cols, bi_vals),
        "ub": (feats["ub"], ub_rows, ub_cols, ub_vals),
    }
    graph_order = ["bi", "ub", "ui"]

    fs = _run(graphs, graph_order)

    agg = {}
    for g in graph_order:
        f1, f2 = fs[g]
        agg[g] = (feats[g] + _l2norm_rows(f1) + _l2norm_rows(f2)) / 3.0

    return np.concatenate(
        [
            agg["ui"][:U],
            agg["ub"][:U],
            agg["bi"][:NB],
            agg["ub"][U : U + NB],
            agg["ui"][U : U + NI],
            agg["bi"][NB : NB + NI],
        ],
        axis=0,
    ).astype(np.float32)
